# revision 16
# baseline (speedup 1.0000x reference)
"""Trainium2 Bass kernel for nn_ExtremeFMLayer.

Math:  out[b,l,d] = sum_{i,j} W[i*F2+j, l] * x0[b,i,d] * x1[b,j,d]
  (B, F1, F2, D, L) = (2048, 64, 64, 16, 16)

Mapping (per core, data-parallel over batch, bd = flattened (b, d) columns):
  stage 1 (PE):   Z[(l,i), bd]   = sum_j W2li[j, (l,i)] * x1t[j, bd]
  stage 2 (DVE):  P[(l,i), bd]   = Z[(l,i), bd] * x0t[i, bd]     (PSUM eviction fused)
  stage 3 (PE):   out[l, bd]     = sum_{(l',i): l'==l} P[(l',i), bd]   (0/1 selector GEMM)

Host sends layout-transformed views only (no compute):
  in64  [64, 1024+4096]  = [ w2li | x1t slice ]   w2li[j, l*64+i] = W[i*64+j, l]
  in128 [128, 128+4096]  = [ wsel | x0t2 slice ]  x0t stacked twice on partitions

The walrus build in this container allows only ONE sync-wait per Matmult,
so inputs are packed into one DMA per partition-width class and tiny dummy
ops absorb each DMA-completion wait before the real compute needs it.
"""

import sys

if "/opt/trn_rl_repo" not in sys.path:
    sys.path.insert(0, "/opt/trn_rl_repo")

import numpy as np

B, F1, F2, D, L = 2048, 64, 64, 16, 16
NCORES = 8
BD = B * D                  # 32768
BDC = BD // NCORES          # 4096 columns per core
NBLK = 8
BLK = BDC // NBLK           # 512
NCHUNK = 8                  # (l,i) chunks of 128 rows

W2_COLS = L * F1            # 1024
SEL_COLS = NCHUNK * L       # 128

_BASS_CACHE: dict = {}


def _build_bass(reps=1):
    from concourse import bass, tile
    from concourse import mybir

    f32 = mybir.dt.float32
    nc = bass.Bass()

    in64_d = nc.declare_dram_parameter("in64", [64, W2_COLS + BDC], f32, isOutput=False)
    in128_d = nc.declare_dram_parameter(
        "in128", [128, SEL_COLS + BDC], f32, isOutput=False
    )
    out_d = nc.declare_dram_parameter("out", [L, BDC], f32, isOutput=True)

    with tile.TileContext(nc) as tc:
        with (
            tc.tile_pool(name="const", bufs=1) as cpool,
            tc.tile_pool(name="xin", bufs=1) as xpool,
            tc.tile_pool(name="prod", bufs=12) as ppool,
            tc.tile_pool(name="outb", bufs=1) as opool,
            tc.tile_pool(name="zpsum", bufs=6, space=bass.MemorySpace.PSUM) as zpool,
            tc.tile_pool(name="opsum", bufs=2, space=bass.MemorySpace.PSUM) as opsum_pool,
        ):
            wselc = cpool.tile([128, SEL_COLS], f32)
            dscr = cpool.tile([16, 1], f32)
            obuf = None

            for rep in range(reps):
                t64 = xpool.tile([64, W2_COLS + BDC], f32, tag="t64")
                t128 = xpool.tile([128, SEL_COLS + BDC], f32, tag="t128")

                nc.sync.dma_start(t64[:], in64_d[:])
                nc.sync.dma_start(t128[:], in128_d[:])

                w2 = t64[:, 0:W2_COLS]
                x1s = t64[:, W2_COLS : W2_COLS + BDC]
                x0s = t128[:, SEL_COLS : SEL_COLS + BDC]

                # DVE absorbs the in128 DMA wait, then (rep 0) stages the
                # selector weights for PE.
                nc.vector.tensor_copy(dscr[:], t128[0:16, 0:1])
                if rep == 0:
                    nc.vector.tensor_copy(wselc[:], t128[:, 0:SEL_COLS])
                    obuf = opool.tile([L, BDC], f32, tag="obuf")
                else:
                    # Absorb the previous rep's output-DMA wait (WAR on
                    # obuf) on DVE before the block loop needs obuf.
                    nc.vector.tensor_copy(obuf[0:16, 0:1], dscr[:])

                for blk in range(NBLK):
                    cs = slice(blk * BLK, (blk + 1) * BLK)
                    opsum = opsum_pool.tile([L, BLK], f32, tag="opsum")
                    for c in range(NCHUNK):
                        zp = zpool.tile([128, BLK], f32)
                        nc.tensor.matmul(
                            zp[:],
                            w2[:, c * 128 : (c + 1) * 128],
                            x1s[:, cs],
                            start=True,
                            stop=True,
                        )
                        prod = ppool.tile([128, BLK], f32, tag="prod")
                        nc.vector.tensor_mul(prod[:], zp[:], x0s[:, cs])
                        nc.tensor.matmul(
                            opsum[:],
                            wselc[:, c * L : (c + 1) * L],
                            prod[:],
                            start=(c == 0),
                            stop=(c == NCHUNK - 1),
                        )
                    nc.vector.tensor_copy(obuf[:, cs], opsum[:])

                nc.sync.dma_start(out_d[:], obuf[:])

    _strip_self_waits(nc)
    return nc


def _strip_self_waits(nc):
    """Drop same-engine semaphore waits (always satisfied: engines complete
    in FIFO order) — this container's walrus allows only one sync-wait per
    data instruction."""
    from bass_rust import SyncInfo

    # Semaphore updated by the final (output) DMA — the only externally
    # visible effect; every other instruction is transitively ordered
    # before it via single waits.
    out_dma_sem = None
    for func in nc.m.functions:
        for blk in func.blocks:
            for inst in blk.instructions:
                if type(inst).__name__ == "InstDMACopy":
                    si = inst.sync_info
                    if si is not None and si.on_update:
                        out_dma_sem = (
                            si.on_update[0].ant_name,
                            si.on_update[0].update_value,
                        )

    for func in nc.m.functions:
        for blk in func.blocks:
            for inst in blk.instructions:
                si = inst.sync_info
                if si is None or not si.on_wait:
                    continue
                eng = str(inst.engine).split(".")[-1]  # e.g. 'DVE', 'PE'
                kept = [w for w in si.on_wait if w.ant_name.split("_")[0] != eng]
                if type(inst).__name__ == "InstDMACopy" and len(kept) > 1:
                    # A compute-engine wait on a reader/writer of the same
                    # buffer transitively covers any DMA-lane wait (the
                    # reader itself waited on that DMA before touching the
                    # data), so keep only the engine waits.
                    eng_waits = [
                        w for w in kept if not w.ant_name.startswith("DMAHW")
                    ]
                    if eng_waits:
                        kept = eng_waits
                if type(inst).__name__ == "InstDrain" and len(kept) > 1:
                    dma_waits = [
                        w
                        for w in kept
                        if out_dma_sem is not None and w.ant_name == out_dma_sem[0]
                    ]
                    kept = dma_waits if dma_waits else kept[-1:]
                if len(kept) > 1:
                    raise RuntimeError(
                        f"instruction {inst.name} still has {len(kept)} waits: "
                        f"{[w.ant_name for w in kept]}"
                    )
                if len(kept) != len(si.on_wait):
                    inst.sync_info = SyncInfo(on_wait=kept, on_update=si.on_update)


def _prep_host(x0, x1, filters):
    x0 = np.asarray(x0, dtype=np.float32)
    x1 = np.asarray(x1, dtype=np.float32)
    w = np.asarray(filters, dtype=np.float32)[0]          # [F1*F2, L]

    # feature-major, (b, d) columns
    x0t = np.ascontiguousarray(x0.transpose(1, 0, 2).reshape(F1, BD))
    x1t = np.ascontiguousarray(x1.transpose(1, 0, 2).reshape(F2, BD))
    x0t2 = np.concatenate([x0t, x0t], axis=0)             # [128, BD]

    # w2li[j, l*F1 + i] = W[i*F2+j, l]
    wf = w.reshape(F1, F2, L)                             # [i, j, l]
    w2li = np.ascontiguousarray(wf.transpose(1, 2, 0).reshape(F2, L * F1))

    wsel = np.zeros((128, SEL_COLS), dtype=np.float32)
    for c in range(NCHUNK):
        for p in range(128):
            l = 2 * c + p // F1
            wsel[p, c * L + l] = 1.0

    return x1t, x0t2, w2li, wsel


def _run(inputs, trace=False):
    from concourse.bass_utils import run_bass_kernel_spmd

    x1t, x0t2, w2li, wsel = _prep_host(
        inputs["x0"], inputs["x1"], inputs["filters"]
    )

    if 1 not in _BASS_CACHE:
        _BASS_CACHE[1] = _build_bass(1)
    nc = _BASS_CACHE[1]

    in_maps = []
    for c in range(NCORES):
        cs = slice(c * BDC, (c + 1) * BDC)
        in64 = np.concatenate([w2li, x1t[:, cs]], axis=1)
        in128 = np.concatenate([wsel, x0t2[:, cs]], axis=1)
        in_maps.append(
            {
                "in64": np.ascontiguousarray(in64),
                "in128": np.ascontiguousarray(in128),
            }
        )

    res = run_bass_kernel_spmd(nc, in_maps, list(range(NCORES)), trace=trace)

    outp = np.concatenate([res.results[c]["out"] for c in range(NCORES)], axis=1)
    # outp[l, b*D+d] -> out[b, l, d]
    out = np.ascontiguousarray(outp.reshape(L, B, D).transpose(1, 0, 2))
    return out, res


def kernel(**inputs):
    out, _ = _run(inputs, trace=False)
    return out


# ----------------------------------------------------------------------
# Benchmarking (test.py only): persistent jitted runner + in-NEFF reps.
# HW time is estimated from the wall-clock slope between reps variants,
# which cancels the per-execute RPC/launch overhead.
# ----------------------------------------------------------------------


def _make_runner(nc, in_maps):
    import jax
    import numpy as np_
    from jax.experimental.shard_map import shard_map
    from jax.sharding import Mesh, NamedSharding, PartitionSpec

    from concourse import bass2jax, mybir

    bass2jax.install_neuronx_cc_hook()

    partition_name = (
        nc.partition_id_tensor.name if nc.partition_id_tensor else None
    )
    in_names, out_names, out_avals, zero_outs = [], [], [], []
    for alloc in nc.m.functions[0].allocations:
        if not isinstance(alloc, mybir.MemoryLocationSet):
            continue
        name = alloc.memorylocations[0].name
        if alloc.kind == "ExternalInput":
            if name != partition_name:
                in_names.append(name)
        elif alloc.kind == "ExternalOutput":
            out_names.append(name)
            shape = tuple(alloc.tensor_shape)
            dtype = mybir.dt.np(alloc.dtype)
            out_avals.append(jax.core.ShapedArray(shape, dtype))
            zero_outs.append(np_.zeros(shape, dtype))

    n_params = len(in_names)
    all_names = in_names + out_names
    if partition_name is not None:
        all_names = all_names + [partition_name]
    donate = tuple(range(n_params, n_params + len(out_names)))

    def _body(*args):
        operands = list(args)
        if partition_name is not None:
            operands.append(bass2jax.partition_id_tensor())
        outs = bass2jax._bass_exec_p.bind(
            *operands,
            out_avals=tuple(out_avals),
            in_names=tuple(all_names),
            out_names=tuple(out_names),
            lowering_input_output_aliases=(),
            sim_require_finite=True,
            sim_require_nnan=True,
            nc=nc,
        )
        return tuple(outs)

    devices = jax.devices()[:NCORES]
    mesh = Mesh(np_.asarray(devices), ("core",))
    spec = PartitionSpec("core")
    in_specs = (spec,) * (n_params + len(out_names))
    out_specs = (spec,) * len(out_names)
    sharded = jax.jit(
        shard_map(
            _body, mesh=mesh, in_specs=in_specs, out_specs=out_specs, check_rep=False
        ),
        donate_argnums=donate,
        keep_unused=True,
    )

    sh = NamedSharding(mesh, spec)
    in_global = [
        jax.device_put(
            np_.concatenate([np_.asarray(m[name]) for m in in_maps], axis=0), sh
        )
        for name in in_names
    ]
    zeros_np = [
        np_.zeros((NCORES * z.shape[0], *z.shape[1:]), z.dtype) for z in zero_outs
    ]

    def call():
        zeros_dev = [jax.device_put(z, sh) for z in zeros_np]
        jax.block_until_ready(zeros_dev)
        import time

        t0 = time.perf_counter()
        out = sharded(*in_global, *zeros_dev)
        jax.block_until_ready(out)
        t1 = time.perf_counter()
        return (t1 - t0), out

    return call


def _minmed(times):
    s = sorted(times)
    return s[0], s[len(s) // 2]


def bench(inputs, reps_pair=(1, 65), n_timed=20):
    x1t, x0t2, w2li, wsel = _prep_host(
        inputs["x0"], inputs["x1"], inputs["filters"]
    )
    in_maps = []
    for c in range(NCORES):
        cs = slice(c * BDC, (c + 1) * BDC)
        in_maps.append(
            {
                "in64": np.ascontiguousarray(
                    np.concatenate([w2li, x1t[:, cs]], axis=1)
                ),
                "in128": np.ascontiguousarray(
                    np.concatenate([wsel, x0t2[:, cs]], axis=1)
                ),
            }
        )

    mins = {}
    raw = {}
    for reps in reps_pair:
        if reps not in _BASS_CACHE:
            _BASS_CACHE[reps] = _build_bass(reps)
        call = _make_runner(_BASS_CACHE[reps], in_maps)
        for _ in range(3):
            call()  # warmup (compile + caches)
        times = [call()[0] for _ in range(n_timed)]
        mins[reps] = min(times)
        raw[reps] = sorted(times)[:5]

    r0, r1 = reps_pair
    per_rep_ns = (mins[r1] - mins[r0]) / (r1 - r0) * 1e9
    return per_rep_ns, mins, raw


# revision 25
# speedup vs baseline: 2.6248x; 2.6248x over previous
"""Trainium2 Bass kernel for nn_ExtremeFMLayer.

Math:  out[b,l,d] = sum_{i,j} W[i*F2+j, l] * x0[b,i,d] * x1[b,j,d]
  (B, F1, F2, D, L) = (2048, 64, 64, 16, 16)

Mapping (per core, data-parallel over batch, bd = flattened (b, d) columns):
  stage 1 (PE):   Z[(l,i), bd]   = sum_j W2li[j, (l,i)] * x1t[j, bd]
                  K=64 row-packed: two chunks run concurrently in array
                  rows 0-63 / 64-127 (tile_position via base_partition).
  stage 2:        P[(l,i), bd]   = Z[(l,i), bd] * x0t[i, bd]
                  split across engines:
                    fused path:  DVE tensor_mul PSUM(fp32) x SBUF(bf16) -> bf16
                    ACT path:    ScalarE copies PSUM -> SBUF bf16, DVE
                                 multiplies in place at 2x bf16 rate
  stage 3 (PE):   out[l, bd]    = selector GEMM over (l,i) chunk partitions
                                  (0/1 weights, accumulated in PSUM)

All inputs ship as ONE bf16 [128, 640 + 2*BDC] tensor per core:
  [ wsel(128) | w2pairs(512) | x1 stacked twice(BDC) | x0 stacked twice(BDC) ]

The walrus build here allows only ONE sync-wait per data instruction; the
structure (single input DMA, DVE absorber, in-place TT, one-semaphore
eviction chains) keeps every instruction at <=1 wait, with a post-pass
stripping provably redundant waits.
"""

import sys

if "/opt/trn_rl_repo" not in sys.path:
    sys.path.insert(0, "/opt/trn_rl_repo")

import numpy as np

B, F1, F2, D, L = 2048, 64, 64, 16, 16
NCORES = 8
BD = B * D                  # 32768
BDC = BD // NCORES          # 4096 columns per core
NBLK = 8
BLK = BDC // NBLK           # 512
NCHUNK = 8                  # (l,i) chunks of 128 rows

SEL_COLS = NCHUNK * L       # 128
W2P_COLS = (NCHUNK // 2) * 128  # 512 (chunk pairs stacked on partitions)
X1_OFF = SEL_COLS + W2P_COLS    # 640
X0_OFF = X1_OFF + BDC
IN_COLS = X0_OFF + BDC

# chunks handled by the fused DVE path (rest go through the ACT-evict path)
FUSED_CHUNKS = (0, 4)

_BASS_CACHE: dict = {}


def _build_bass(reps=1):
    from concourse import bass, tile
    from concourse import mybir

    f32 = mybir.dt.float32
    bf16 = mybir.dt.bfloat16
    nc = bass.Bass()

    in_d = nc.declare_dram_parameter("inp", [128, IN_COLS], bf16, isOutput=False)
    out_d = nc.declare_dram_parameter("out", [L, BDC], f32, isOutput=True)

    with tile.TileContext(nc) as tc:
        with (
            tc.tile_pool(name="const", bufs=1) as cpool,
            tc.tile_pool(name="xin", bufs=1) as xpool,
            tc.tile_pool(name="prod", bufs=12) as ppool,
            tc.tile_pool(name="outb", bufs=1) as opool,
            tc.tile_pool(name="zpsum", bufs=6, space=bass.MemorySpace.PSUM) as zpool,
            tc.tile_pool(name="opsum", bufs=2, space=bass.MemorySpace.PSUM) as opsum_pool,
        ):
            dscr = cpool.tile([16, 1], bf16)
            obuf = None

            for rep in range(reps):
                t = xpool.tile([128, IN_COLS], bf16, tag="t")
                nc.sync.dma_start(t[:], in_d[:])

                wsel = t[:, 0:SEL_COLS]
                w2p = t[:, SEL_COLS:X1_OFF]
                x1d = t[:, X1_OFF : X1_OFF + BDC]
                x0d = t[:, X0_OFF : X0_OFF + BDC]

                # DVE absorbs the input-DMA wait.
                nc.vector.tensor_copy(dscr[:], t[0:16, 0:1])
                if rep == 0:
                    obuf = opool.tile([L, BDC], f32, tag="obuf")
                else:
                    # Absorb the previous rep's output-DMA wait (WAR on
                    # obuf) on DVE before the block loop needs obuf.
                    nc.vector.tensor_copy(obuf[0:16, 0:1], dscr[:])

                for blk in range(NBLK):
                    cs = slice(blk * BLK, (blk + 1) * BLK)
                    opsum = opsum_pool.tile([L, BLK], f32, tag="opsum")
                    for c in range(NCHUNK):
                        zp = zpool.tile([128, BLK], f32)
                        pb = 64 * (c % 2)
                        nc.tensor.matmul(
                            zp[:],
                            w2p[pb : pb + 64, (c // 2) * 128 : (c // 2 + 1) * 128],
                            x1d[pb : pb + 64, cs],
                            start=True,
                            stop=True,
                        )
                        if c in FUSED_CHUNKS:
                            prod = ppool.tile([128, BLK], bf16, tag="prod")
                            nc.vector.tensor_mul(prod[:], zp[:], x0d[:, cs])
                        else:
                            prod = ppool.tile([128, BLK], bf16, tag="prod")
                            nc.scalar.copy(prod[:], zp[:])
                            nc.vector.tensor_mul(prod[:], prod[:], x0d[:, cs])
                        nc.tensor.matmul(
                            opsum[:],
                            wsel[:, c * L : (c + 1) * L],
                            prod[:],
                            start=(c == 0),
                            stop=(c == NCHUNK - 1),
                        )
                    nc.vector.tensor_copy(obuf[:, cs], opsum[:])

                nc.sync.dma_start(out_d[:], obuf[:])

    _strip_self_waits(nc)
    return nc


def _strip_self_waits(nc):
    """Transitively minimize semaphore waits (this container's walrus allows
    only ONE sync-wait per data instruction).

    Tile emits per-engine-minimal waits but does not track that syncing on
    engine X also conveys everything X had itself waited on.  We recompute a
    conservative happens-before: walk instructions in BIR order (a valid
    topological/issue order), maintain per-engine knowledge as a vector
    clock over semaphore values, and record, per semaphore value, the
    (joined) knowledge implied by the updating instruction's completion.
    A wait that is covered by engine knowledge plus the other kept waits is
    dropped."""
    from bass_rust import SyncInfo

    def join(a, b):
        for k, v in b.items():
            if a.get(k, 0) < v:
                a[k] = v
        return a

    def covers(k, sem, val):
        return k.get(sem, 0) >= val

    sem_cum: dict = {}
    # per-sem running joined knowledge along its event sequence:
    # list of (cum_value, knowledge_dict_at_or_before_this_value)
    sem_events: dict = {}
    engine_know: dict = {}

    # Semaphores that are ever decremented/reset (barrier gather sems) are
    # not monotone — never reason about them, never drop their waits.
    nonmono = set()
    for func in nc.m.functions:
        for blk in func.blocks:
            for inst in blk.instructions:
                si = inst.sync_info
                if si is None:
                    continue
                for upd in si.on_update:
                    if upd.update_mode not in ("sem-inc", "sem-add-imm"):
                        nonmono.add(upd.ant_name)

    def wait_knowledge(sem, val):
        """Knowledge implied by observing sem >= val."""
        k = {sem: val}
        events = sem_events.get(sem)
        if not events:
            return k
        # join knowledge of all events with cum <= observed value is already
        # accumulated (running join); take the latest event with cum <= val
        # ... but sem >= val implies all events up to the FIRST event with
        # cum >= val have completed.
        best = None
        for cum, kn in events:
            if cum >= val:
                best = kn
                break
        if best is None:
            best = events[-1][1]
        return join(dict(best), k)

    for func in nc.m.functions:
        for blk in func.blocks:
            for inst in blk.instructions:
                eng = str(inst.engine).split(".")[-1]
                know = engine_know.setdefault(eng, {})
                si = inst.sync_info
                waits = list(si.on_wait) if si is not None else []
                updates = list(si.on_update) if si is not None else []

                if waits:
                    wait_ks = [
                        {} if w.ant_name in nonmono
                        else wait_knowledge(w.ant_name, w.wait_value)
                        for w in waits
                    ]
                    # keep strongest-first waits not covered by engine
                    # knowledge + already-kept waits
                    order = sorted(range(len(waits)), key=lambda i: -len(wait_ks[i]))
                    kept, kept_ks = [], []
                    for i in order:
                        if waits[i].ant_name in nonmono:
                            kept.append(waits[i])
                            kept_ks.append(wait_ks[i])
                            continue
                        base = dict(know)
                        for kk in kept_ks:
                            join(base, kk)
                        if covers(base, waits[i].ant_name, waits[i].wait_value):
                            continue
                        kept.append(waits[i])
                        kept_ks.append(wait_ks[i])
                    # all original waits' knowledge is valid here (each
                    # condition holds once the kept set is satisfied)
                    for kk in wait_ks:
                        join(know, kk)
                    if len(kept) > 1:
                        raise RuntimeError(
                            f"instruction {inst.name} still has {len(kept)} "
                            f"waits: {[w.ant_name for w in kept]} "
                            f"({str(inst)[:220]})"
                        )
                    if len(kept) != len(waits):
                        inst.sync_info = SyncInfo(
                            on_wait=kept, on_update=updates
                        )

                for upd in updates:
                    s = upd.ant_name
                    if s in nonmono:
                        continue
                    sem_cum[s] = sem_cum.get(s, 0) + upd.update_value
                    post = dict(know)
                    post[s] = sem_cum[s]
                    events = sem_events.setdefault(s, [])
                    if events:
                        post = join(dict(events[-1][1]), post)
                    events.append((sem_cum[s], post))
                    # Same-engine completions are ordered: the engine's next
                    # instruction may rely on this one having finished —
                    # but ONLY for the engine's own semaphore (DMA-lane sems
                    # fire asynchronously at transfer completion).
                    if s.split("_")[0] == eng:
                        if know.get(s, 0) < sem_cum[s]:
                            know[s] = sem_cum[s]


def _prep_host(x0, x1, filters):
    import ml_dtypes

    bf16 = ml_dtypes.bfloat16

    x0 = np.asarray(x0, dtype=np.float32)
    x1 = np.asarray(x1, dtype=np.float32)
    w = np.asarray(filters, dtype=np.float32)[0]          # [F1*F2, L]

    # feature-major, (b, d) columns
    x0t = x0.transpose(1, 0, 2).reshape(F1, BD)
    x1t = x1.transpose(1, 0, 2).reshape(F2, BD)
    x0d = np.concatenate([x0t, x0t], axis=0).astype(bf16)  # [128, BD]
    x1d = np.concatenate([x1t, x1t], axis=0).astype(bf16)  # [128, BD]

    # w2li[j, l*F1 + i] = W[i*F2+j, l]
    wf = w.reshape(F1, F2, L)                             # [i, j, l]
    w2li = wf.transpose(1, 2, 0).reshape(F2, L * F1)      # [j, (l,i)]

    # chunk pairs stacked on partitions: [128, 4, 128]
    w2pair = np.empty((128, NCHUNK // 2, 128), dtype=np.float32)
    for cp in range(NCHUNK // 2):
        w2pair[0:64, cp, :] = w2li[:, (2 * cp) * 128 : (2 * cp + 1) * 128]
        w2pair[64:128, cp, :] = w2li[:, (2 * cp + 1) * 128 : (2 * cp + 2) * 128]
    w2pair = w2pair.reshape(128, W2P_COLS).astype(bf16)

    wsel = np.zeros((128, SEL_COLS), dtype=np.float32)
    for c in range(NCHUNK):
        for p in range(128):
            l = 2 * c + p // F1
            wsel[p, c * L + l] = 1.0
    wsel = wsel.astype(bf16)

    return wsel, w2pair, x1d, x0d


def _core_in_maps(inputs):
    wsel, w2pair, x1d, x0d = _prep_host(
        inputs["x0"], inputs["x1"], inputs["filters"]
    )
    in_maps = []
    for c in range(NCORES):
        cs = slice(c * BDC, (c + 1) * BDC)
        inp = np.concatenate([wsel, w2pair, x1d[:, cs], x0d[:, cs]], axis=1)
        in_maps.append({"inp": np.ascontiguousarray(inp)})
    return in_maps


def _run(inputs, trace=False):
    from concourse.bass_utils import run_bass_kernel_spmd

    if 1 not in _BASS_CACHE:
        _BASS_CACHE[1] = _build_bass(1)
    nc = _BASS_CACHE[1]

    in_maps = _core_in_maps(inputs)
    res = run_bass_kernel_spmd(nc, in_maps, list(range(NCORES)), trace=trace)

    outp = np.concatenate([res.results[c]["out"] for c in range(NCORES)], axis=1)
    # outp[l, b*D+d] -> out[b, l, d]
    out = np.ascontiguousarray(outp.reshape(L, B, D).transpose(1, 0, 2))
    return out, res


def kernel(**inputs):
    out, _ = _run(inputs, trace=False)
    return out


# ----------------------------------------------------------------------
# Benchmarking (test.py only): persistent jitted runner + in-NEFF reps.
# HW time is estimated from the wall-clock slope between reps variants,
# which cancels the per-execute RPC/launch overhead.
# ----------------------------------------------------------------------


def _make_runner(nc, in_maps):
    import jax
    import numpy as np_
    from jax.experimental.shard_map import shard_map
    from jax.sharding import Mesh, NamedSharding, PartitionSpec

    from concourse import bass2jax, mybir

    bass2jax.install_neuronx_cc_hook()

    partition_name = (
        nc.partition_id_tensor.name if nc.partition_id_tensor else None
    )
    in_names, out_names, out_avals, zero_outs = [], [], [], []
    for alloc in nc.m.functions[0].allocations:
        if not isinstance(alloc, mybir.MemoryLocationSet):
            continue
        name = alloc.memorylocations[0].name
        if alloc.kind == "ExternalInput":
            if name != partition_name:
                in_names.append(name)
        elif alloc.kind == "ExternalOutput":
            out_names.append(name)
            shape = tuple(alloc.tensor_shape)
            dtype = mybir.dt.np(alloc.dtype)
            out_avals.append(jax.core.ShapedArray(shape, dtype))
            zero_outs.append(np_.zeros(shape, dtype))

    n_params = len(in_names)
    all_names = in_names + out_names
    if partition_name is not None:
        all_names = all_names + [partition_name]
    donate = tuple(range(n_params, n_params + len(out_names)))

    def _body(*args):
        operands = list(args)
        if partition_name is not None:
            operands.append(bass2jax.partition_id_tensor())
        outs = bass2jax._bass_exec_p.bind(
            *operands,
            out_avals=tuple(out_avals),
            in_names=tuple(all_names),
            out_names=tuple(out_names),
            lowering_input_output_aliases=(),
            sim_require_finite=True,
            sim_require_nnan=True,
            nc=nc,
        )
        return tuple(outs)

    devices = jax.devices()[:NCORES]
    mesh = Mesh(np_.asarray(devices), ("core",))
    spec = PartitionSpec("core")
    in_specs = (spec,) * (n_params + len(out_names))
    out_specs = (spec,) * len(out_names)
    sharded = jax.jit(
        shard_map(
            _body, mesh=mesh, in_specs=in_specs, out_specs=out_specs, check_rep=False
        ),
        donate_argnums=donate,
        keep_unused=True,
    )

    sh = NamedSharding(mesh, spec)
    in_global = [
        jax.device_put(
            np_.concatenate([np_.asarray(m[name]) for m in in_maps], axis=0), sh
        )
        for name in in_names
    ]
    zeros_np = [
        np_.zeros((NCORES * z.shape[0], *z.shape[1:]), z.dtype) for z in zero_outs
    ]

    def call():
        zeros_dev = [jax.device_put(z, sh) for z in zeros_np]
        jax.block_until_ready(zeros_dev)
        import time

        t0 = time.perf_counter()
        out = sharded(*in_global, *zeros_dev)
        jax.block_until_ready(out)
        t1 = time.perf_counter()
        return (t1 - t0), out

    return call


def bench(inputs, reps_pair=(1, 65), n_timed=20):
    in_maps = _core_in_maps(inputs)

    mins = {}
    raw = {}
    for reps in reps_pair:
        if reps not in _BASS_CACHE:
            _BASS_CACHE[reps] = _build_bass(reps)
        call = _make_runner(_BASS_CACHE[reps], in_maps)
        for _ in range(3):
            call()  # warmup (compile + caches)
        times = [call()[0] for _ in range(n_timed)]
        mins[reps] = min(times)
        raw[reps] = sorted(times)[:5]

    r0, r1 = reps_pair
    per_rep_ns = (mins[r1] - mins[r0]) / (r1 - r0) * 1e9
    return per_rep_ns, mins, raw


# revision 29
# speedup vs baseline: 4.0154x; 1.5298x over previous
"""Trainium2 Bass kernel for nn_ExtremeFMLayer.

Math:  out[b,l,d] = sum_{i,j} W[i*F2+j, l] * x0[b,i,d] * x1[b,j,d]
  (B, F1, F2, D, L) = (2048, 64, 64, 16, 16)

Mapping (per core, data-parallel over batch, bd = flattened (b, d) columns):
  stage 1 (PE):   Z[(l,i), bd]   = sum_j W2li[j, (l,i)] * x1t[j, bd]
                  K=64 row-packed: two chunks run concurrently in array
                  rows 0-63 / 64-127 (tile_position via base_partition).
  stage 2:        P[(l,i), bd]   = Z[(l,i), bd] * x0t[i, bd]
                  split across engines:
                    fused path:  DVE tensor_mul PSUM(fp32) x SBUF(bf16) -> bf16
                    ACT path:    ScalarE copies PSUM -> SBUF bf16, DVE
                                 multiplies in place at 2x bf16 rate
  stage 3 (PE):   out[l, bd]    = selector GEMM over (l,i) chunk partitions
                                  (0/1 weights, accumulated in PSUM)

All inputs ship as ONE bf16 [128, 640 + 2*BDC] tensor per core:
  [ wsel(128) | w2pairs(512) | x1 stacked twice(BDC) | x0 stacked twice(BDC) ]

The walrus build here allows only ONE sync-wait per data instruction; the
structure (single input DMA, DVE absorber, in-place TT, one-semaphore
eviction chains) keeps every instruction at <=1 wait, with a post-pass
stripping provably redundant waits.
"""

import sys

if "/opt/trn_rl_repo" not in sys.path:
    sys.path.insert(0, "/opt/trn_rl_repo")

import numpy as np

B, F1, F2, D, L = 2048, 64, 64, 16, 16
NCORES = 8
BD = B * D                  # 32768
BDC = BD // NCORES          # 4096 columns per core
NBLK = 8
BLK = BDC // NBLK           # 512
NCHUNK = 8                  # (l,i) chunks of 128 rows

SEL_COLS = NCHUNK * L       # 128
W2P_COLS = (NCHUNK // 2) * 128  # 512 (chunk pairs stacked on partitions)
X1_OFF = SEL_COLS + W2P_COLS    # 640
X0_OFF = X1_OFF + BDC
IN_COLS = X0_OFF + BDC

# chunk-pairs handled by the fused DVE path, per block parity (engine balance)
FUSED_PAIRS_EVEN = (0,)
FUSED_PAIRS_ODD = ()

_BASS_CACHE: dict = {}


def _build_bass(reps=1):
    from concourse import bass, tile
    from concourse import mybir

    f32 = mybir.dt.float32
    bf16 = mybir.dt.bfloat16
    nc = bass.Bass()

    in_d = nc.declare_dram_parameter("inp", [128, IN_COLS], bf16, isOutput=False)
    out_d = nc.declare_dram_parameter("out", [L, BDC], f32, isOutput=True)

    with tile.TileContext(nc) as tc:
        with (
            tc.tile_pool(name="const", bufs=1) as cpool,
            tc.tile_pool(name="xin", bufs=1) as xpool,
            tc.tile_pool(name="prod", bufs=6) as ppool,
            tc.tile_pool(name="outb", bufs=1) as opool,
            tc.tile_pool(name="zpsum", bufs=3, space=bass.MemorySpace.PSUM) as zpool,
            tc.tile_pool(name="opsum", bufs=2, space=bass.MemorySpace.PSUM) as opsum_pool,
        ):
            dscr = cpool.tile([16, 1], bf16)
            obuf = None

            for rep in range(reps):
                t = xpool.tile([128, IN_COLS], bf16, tag="t")
                nc.sync.dma_start(t[:], in_d[:])

                wsel = t[:, 0:SEL_COLS]
                w2p = t[:, SEL_COLS:X1_OFF]
                x1d = t[:, X1_OFF : X1_OFF + BDC]
                x0d = t[:, X0_OFF : X0_OFF + BDC]

                # DVE absorbs the input-DMA wait.
                nc.vector.tensor_copy(dscr[:], t[0:16, 0:1])
                if rep == 0:
                    obuf = opool.tile([L, BDC], f32, tag="obuf")
                else:
                    # Absorb the previous rep's output-DMA wait (WAR on
                    # obuf) on DVE before the block loop needs obuf.
                    nc.vector.tensor_copy(obuf[0:16, 0:1], dscr[:])

                for blk in range(NBLK):
                    cs = slice(blk * BLK, (blk + 1) * BLK)
                    fused_pairs = (
                        FUSED_PAIRS_EVEN if blk % 2 == 0 else FUSED_PAIRS_ODD
                    )
                    x0b = (
                        x0d[:, cs]
                        .rearrange("p (a b) -> p a b", a=1)
                        .to_broadcast((128, 2, BLK))
                    )
                    opsum = opsum_pool.tile([L, BLK], f32, tag="opsum")
                    for cp in range(NCHUNK // 2):
                        zp = zpool.tile([128, 2 * BLK], f32)
                        nc.tensor.matmul(
                            zp[:, 0:BLK],
                            w2p[0:64, cp * 128 : (cp + 1) * 128],
                            x1d[0:64, cs],
                            start=True,
                            stop=True,
                        )
                        nc.tensor.matmul(
                            zp[:, BLK : 2 * BLK],
                            w2p[64:128, cp * 128 : (cp + 1) * 128],
                            x1d[64:128, cs],
                            start=True,
                            stop=True,
                        )
                        prod = ppool.tile([128, 2 * BLK], bf16, tag="prod")
                        pv = prod[:].rearrange("p (a b) -> p a b", a=2)
                        if cp in fused_pairs:
                            nc.vector.tensor_tensor(
                                pv,
                                zp[:].rearrange("p (a b) -> p a b", a=2),
                                x0b,
                                op=mybir.AluOpType.mult,
                            )
                        else:
                            nc.scalar.copy(prod[:], zp[:])
                            nc.vector.tensor_tensor(
                                pv, pv, x0b, op=mybir.AluOpType.mult
                            )
                        for h in range(2):
                            c = 2 * cp + h
                            nc.tensor.matmul(
                                opsum[:],
                                wsel[:, c * L : (c + 1) * L],
                                prod[:, h * BLK : (h + 1) * BLK],
                                start=(c == 0),
                                stop=(c == NCHUNK - 1),
                            )
                    nc.vector.tensor_copy(obuf[:, cs], opsum[:])

                nc.sync.dma_start(out_d[:], obuf[:])

    _strip_self_waits(nc)
    return nc


def _strip_self_waits(nc):
    """Transitively minimize semaphore waits (this container's walrus allows
    only ONE sync-wait per data instruction).

    Tile emits per-engine-minimal waits but does not track that syncing on
    engine X also conveys everything X had itself waited on.  We recompute a
    conservative happens-before: walk instructions in BIR order (a valid
    topological/issue order), maintain per-engine knowledge as a vector
    clock over semaphore values, and record, per semaphore value, the
    (joined) knowledge implied by the updating instruction's completion.
    A wait that is covered by engine knowledge plus the other kept waits is
    dropped."""
    from bass_rust import SyncInfo

    def join(a, b):
        for k, v in b.items():
            if a.get(k, 0) < v:
                a[k] = v
        return a

    def covers(k, sem, val):
        return k.get(sem, 0) >= val

    sem_cum: dict = {}
    # per-sem running joined knowledge along its event sequence:
    # list of (cum_value, knowledge_dict_at_or_before_this_value)
    sem_events: dict = {}
    engine_know: dict = {}

    # Semaphores that are ever decremented/reset (barrier gather sems) are
    # not monotone — never reason about them, never drop their waits.
    nonmono = set()
    for func in nc.m.functions:
        for blk in func.blocks:
            for inst in blk.instructions:
                si = inst.sync_info
                if si is None:
                    continue
                for upd in si.on_update:
                    if upd.update_mode not in ("sem-inc", "sem-add-imm"):
                        nonmono.add(upd.ant_name)

    def wait_knowledge(sem, val):
        """Knowledge implied by observing sem >= val."""
        k = {sem: val}
        events = sem_events.get(sem)
        if not events:
            return k
        # join knowledge of all events with cum <= observed value is already
        # accumulated (running join); take the latest event with cum <= val
        # ... but sem >= val implies all events up to the FIRST event with
        # cum >= val have completed.
        best = None
        for cum, kn in events:
            if cum >= val:
                best = kn
                break
        if best is None:
            best = events[-1][1]
        return join(dict(best), k)

    for func in nc.m.functions:
        for blk in func.blocks:
            for inst in blk.instructions:
                eng = str(inst.engine).split(".")[-1]
                know = engine_know.setdefault(eng, {})
                si = inst.sync_info
                waits = list(si.on_wait) if si is not None else []
                updates = list(si.on_update) if si is not None else []

                if waits:
                    wait_ks = [
                        {} if w.ant_name in nonmono
                        else wait_knowledge(w.ant_name, w.wait_value)
                        for w in waits
                    ]
                    # keep strongest-first waits not covered by engine
                    # knowledge + already-kept waits
                    order = sorted(range(len(waits)), key=lambda i: -len(wait_ks[i]))
                    kept, kept_ks = [], []
                    for i in order:
                        if waits[i].ant_name in nonmono:
                            kept.append(waits[i])
                            kept_ks.append(wait_ks[i])
                            continue
                        base = dict(know)
                        for kk in kept_ks:
                            join(base, kk)
                        if covers(base, waits[i].ant_name, waits[i].wait_value):
                            continue
                        kept.append(waits[i])
                        kept_ks.append(wait_ks[i])
                    # elimination pass: a kept wait may be covered by the
                    # union of the OTHER kept waits' knowledge
                    changed = True
                    while changed and len(kept) > 1:
                        changed = False
                        for i in range(len(kept)):
                            if kept[i].ant_name in nonmono:
                                continue
                            base = dict(know)
                            for j in range(len(kept)):
                                if j != i:
                                    join(base, kept_ks[j])
                            if covers(base, kept[i].ant_name, kept[i].wait_value):
                                kept.pop(i)
                                kept_ks.pop(i)
                                changed = True
                                break
                    # all original waits' knowledge is valid here (each
                    # condition holds once the kept set is satisfied)
                    for kk in wait_ks:
                        join(know, kk)
                    if len(kept) > 1:
                        raise RuntimeError(
                            f"instruction {inst.name} still has {len(kept)} "
                            f"waits: {[w.ant_name for w in kept]} "
                            f"({str(inst)[:220]})"
                        )
                    if len(kept) != len(waits):
                        inst.sync_info = SyncInfo(
                            on_wait=kept, on_update=updates
                        )

                for upd in updates:
                    s = upd.ant_name
                    if s in nonmono:
                        continue
                    sem_cum[s] = sem_cum.get(s, 0) + upd.update_value
                    post = dict(know)
                    post[s] = sem_cum[s]
                    events = sem_events.setdefault(s, [])
                    if events:
                        post = join(dict(events[-1][1]), post)
                    events.append((sem_cum[s], post))
                    # Same-engine completions are ordered: the engine's next
                    # instruction may rely on this one having finished —
                    # but ONLY for the engine's own semaphore (DMA-lane sems
                    # fire asynchronously at transfer completion).
                    if s.split("_")[0] == eng:
                        if know.get(s, 0) < sem_cum[s]:
                            know[s] = sem_cum[s]


def _prep_host(x0, x1, filters):
    import ml_dtypes

    bf16 = ml_dtypes.bfloat16

    x0 = np.asarray(x0, dtype=np.float32)
    x1 = np.asarray(x1, dtype=np.float32)
    w = np.asarray(filters, dtype=np.float32)[0]          # [F1*F2, L]

    # feature-major, (b, d) columns
    x0t = x0.transpose(1, 0, 2).reshape(F1, BD)
    x1t = x1.transpose(1, 0, 2).reshape(F2, BD)
    x0d = np.concatenate([x0t, x0t], axis=0).astype(bf16)  # [128, BD]
    x1d = np.concatenate([x1t, x1t], axis=0).astype(bf16)  # [128, BD]

    # w2li[j, l*F1 + i] = W[i*F2+j, l]
    wf = w.reshape(F1, F2, L)                             # [i, j, l]
    w2li = wf.transpose(1, 2, 0).reshape(F2, L * F1)      # [j, (l,i)]

    # chunk pairs stacked on partitions: [128, 4, 128]
    w2pair = np.empty((128, NCHUNK // 2, 128), dtype=np.float32)
    for cp in range(NCHUNK // 2):
        w2pair[0:64, cp, :] = w2li[:, (2 * cp) * 128 : (2 * cp + 1) * 128]
        w2pair[64:128, cp, :] = w2li[:, (2 * cp + 1) * 128 : (2 * cp + 2) * 128]
    w2pair = w2pair.reshape(128, W2P_COLS).astype(bf16)

    wsel = np.zeros((128, SEL_COLS), dtype=np.float32)
    for c in range(NCHUNK):
        for p in range(128):
            l = 2 * c + p // F1
            wsel[p, c * L + l] = 1.0
    wsel = wsel.astype(bf16)

    return wsel, w2pair, x1d, x0d


def _core_in_maps(inputs):
    wsel, w2pair, x1d, x0d = _prep_host(
        inputs["x0"], inputs["x1"], inputs["filters"]
    )
    in_maps = []
    for c in range(NCORES):
        cs = slice(c * BDC, (c + 1) * BDC)
        inp = np.concatenate([wsel, w2pair, x1d[:, cs], x0d[:, cs]], axis=1)
        in_maps.append({"inp": np.ascontiguousarray(inp)})
    return in_maps


def _run(inputs, trace=False):
    from concourse.bass_utils import run_bass_kernel_spmd

    if 1 not in _BASS_CACHE:
        _BASS_CACHE[1] = _build_bass(1)
    nc = _BASS_CACHE[1]

    in_maps = _core_in_maps(inputs)
    res = run_bass_kernel_spmd(nc, in_maps, list(range(NCORES)), trace=trace)

    outp = np.concatenate([res.results[c]["out"] for c in range(NCORES)], axis=1)
    # outp[l, b*D+d] -> out[b, l, d]
    out = np.ascontiguousarray(outp.reshape(L, B, D).transpose(1, 0, 2))
    return out, res


def kernel(**inputs):
    out, _ = _run(inputs, trace=False)
    return out


# ----------------------------------------------------------------------
# Benchmarking (test.py only): persistent jitted runner + in-NEFF reps.
# HW time is estimated from the wall-clock slope between reps variants,
# which cancels the per-execute RPC/launch overhead.
# ----------------------------------------------------------------------


def _make_runner(nc, in_maps):
    import jax
    import numpy as np_
    from jax.experimental.shard_map import shard_map
    from jax.sharding import Mesh, NamedSharding, PartitionSpec

    from concourse import bass2jax, mybir

    bass2jax.install_neuronx_cc_hook()

    partition_name = (
        nc.partition_id_tensor.name if nc.partition_id_tensor else None
    )
    in_names, out_names, out_avals, zero_outs = [], [], [], []
    for alloc in nc.m.functions[0].allocations:
        if not isinstance(alloc, mybir.MemoryLocationSet):
            continue
        name = alloc.memorylocations[0].name
        if alloc.kind == "ExternalInput":
            if name != partition_name:
                in_names.append(name)
        elif alloc.kind == "ExternalOutput":
            out_names.append(name)
            shape = tuple(alloc.tensor_shape)
            dtype = mybir.dt.np(alloc.dtype)
            out_avals.append(jax.core.ShapedArray(shape, dtype))
            zero_outs.append(np_.zeros(shape, dtype))

    n_params = len(in_names)
    all_names = in_names + out_names
    if partition_name is not None:
        all_names = all_names + [partition_name]
    donate = tuple(range(n_params, n_params + len(out_names)))

    def _body(*args):
        operands = list(args)
        if partition_name is not None:
            operands.append(bass2jax.partition_id_tensor())
        outs = bass2jax._bass_exec_p.bind(
            *operands,
            out_avals=tuple(out_avals),
            in_names=tuple(all_names),
            out_names=tuple(out_names),
            lowering_input_output_aliases=(),
            sim_require_finite=True,
            sim_require_nnan=True,
            nc=nc,
        )
        return tuple(outs)

    devices = jax.devices()[:NCORES]
    mesh = Mesh(np_.asarray(devices), ("core",))
    spec = PartitionSpec("core")
    in_specs = (spec,) * (n_params + len(out_names))
    out_specs = (spec,) * len(out_names)
    sharded = jax.jit(
        shard_map(
            _body, mesh=mesh, in_specs=in_specs, out_specs=out_specs, check_rep=False
        ),
        donate_argnums=donate,
        keep_unused=True,
    )

    sh = NamedSharding(mesh, spec)
    in_global = [
        jax.device_put(
            np_.concatenate([np_.asarray(m[name]) for m in in_maps], axis=0), sh
        )
        for name in in_names
    ]
    zeros_np = [
        np_.zeros((NCORES * z.shape[0], *z.shape[1:]), z.dtype) for z in zero_outs
    ]

    def call():
        zeros_dev = [jax.device_put(z, sh) for z in zeros_np]
        jax.block_until_ready(zeros_dev)
        import time

        t0 = time.perf_counter()
        out = sharded(*in_global, *zeros_dev)
        jax.block_until_ready(out)
        t1 = time.perf_counter()
        return (t1 - t0), out

    return call


def bench(inputs, reps_pair=(1, 65), n_timed=20):
    in_maps = _core_in_maps(inputs)

    mins = {}
    raw = {}
    for reps in reps_pair:
        if reps not in _BASS_CACHE:
            _BASS_CACHE[reps] = _build_bass(reps)
        call = _make_runner(_BASS_CACHE[reps], in_maps)
        for _ in range(3):
            call()  # warmup (compile + caches)
        times = [call()[0] for _ in range(n_timed)]
        mins[reps] = min(times)
        raw[reps] = sorted(times)[:5]

    r0, r1 = reps_pair
    per_rep_ns = (mins[r1] - mins[r0]) / (r1 - r0) * 1e9
    return per_rep_ns, mins, raw


# revision 39
# speedup vs baseline: 4.4994x; 1.1205x over previous
"""Trainium2 Bass kernel for nn_ExtremeFMLayer.

Math:  out[b,l,d] = sum_{i,j} W[i*F2+j, l] * x0[b,i,d] * x1[b,j,d]
  (B, F1, F2, D, L) = (2048, 64, 64, 16, 16)

Mapping (per core, data-parallel over batch, bd = flattened (b, d) columns):
  stage 1 (PE):   Z[(l,i), bd]   = sum_j W2li[j, (l,i)] * x1t[j, bd]
                  K=64 row-packed: two chunks run concurrently in array
                  rows 0-63 / 64-127 (tile_position via base_partition).
  stage 2:        P[(l,i), bd]   = Z[(l,i), bd] * x0t[i, bd]
                  split across engines:
                    fused path:  DVE tensor_mul PSUM(fp32) x SBUF(bf16) -> bf16
                    ACT path:    ScalarE copies PSUM -> SBUF bf16, DVE
                                 multiplies in place at 2x bf16 rate
  stage 3 (PE):   out[l, bd]    = selector GEMM over (l,i) chunk partitions
                                  (0/1 weights, accumulated in PSUM)

All inputs ship as ONE bf16 [128, 640 + 2*BDC] tensor per core:
  [ wsel(128) | w2pairs(512) | x1 stacked twice(BDC) | x0 stacked twice(BDC) ]

The walrus build here allows only ONE sync-wait per data instruction; the
structure (single input DMA, DVE absorber, in-place TT, one-semaphore
eviction chains) keeps every instruction at <=1 wait, with a post-pass
stripping provably redundant waits.
"""

import sys

if "/opt/trn_rl_repo" not in sys.path:
    sys.path.insert(0, "/opt/trn_rl_repo")

import numpy as np

B, F1, F2, D, L = 2048, 64, 64, 16, 16
NCORES = 8
BD = B * D                  # 32768
BDC = BD // NCORES          # 4096 columns per core
NBLK = 8
BLK = BDC // NBLK           # 512
NCHUNK = 8                  # (l,i) chunks of 128 rows

SEL_COLS = NCHUNK * L       # 128
W2P_COLS = (NCHUNK // 2) * 128  # 512 (chunk pairs stacked on partitions)
W_COLS = SEL_COLS + W2P_COLS    # 640
NGRP = 4                    # data shipped as 4 DMAs of 2 blocks each
GRP_COLS = 2 * BLK * 2      # x1 pair-of-blocks + x0 pair-of-blocks = 2048
IN_COLS = W_COLS + NGRP * GRP_COLS

# chunk-pairs handled by the fused DVE path, per block parity (engine balance)
FUSED_PAIRS_EVEN = (0,)
FUSED_PAIRS_ODD = ()

_BASS_CACHE: dict = {}

# Bumped on every kernel change: the persistent NEFF compile cache keys on
# the HLO (shapes/names only, not BIR contents), so a shape-unique dummy
# input is needed to keep kernel variants from silently reusing each
# other's NEFFs.
VERSION = 40


def _build_bass(reps=1):
    from concourse import bass, tile
    from concourse import mybir

    f32 = mybir.dt.float32
    bf16 = mybir.dt.bfloat16
    nc = bass.Bass()

    in_d = nc.declare_dram_parameter("inp", [128, IN_COLS], bf16, isOutput=False)
    nc.declare_dram_parameter("ver", [1, VERSION * 1000 + reps], f32, isOutput=False)
    out_d = nc.declare_dram_parameter("out", [L, BDC], f32, isOutput=True)

    with tile.TileContext(nc) as tc:
        with (
            tc.tile_pool(name="const", bufs=1) as cpool,
            tc.tile_pool(name="xin", bufs=2) as xpool,
            tc.tile_pool(name="prod", bufs=6) as ppool,
            tc.tile_pool(name="outb", bufs=1) as opool,
            tc.tile_pool(name="zpsum", bufs=3, space=bass.MemorySpace.PSUM) as zpool,
            tc.tile_pool(name="opsum", bufs=2, space=bass.MemorySpace.PSUM) as opsum_pool,
        ):
            dscr = cpool.tile([16, 512], bf16)
            obuf = None
            absorb_idx = [0]

            def absorb(col, pe=True, dve=True):
                # Tiny ops that make PE/DVE observe a DMA-completion wait
                # early, so real instructions carry at most one wait.
                # PE: a dummy LDWEIGHTS (no PSUM write, overwritten by the
                # next matmul's own weight load).  DVE: a 1-column copy to
                # a distinct dscr column (race-free).
                k = absorb_idx[0]
                absorb_idx[0] += 1
                if pe:
                    nc.tensor.ldweights(t[0:1, col : col + 1])
                if dve:
                    nc.vector.tensor_copy(
                        dscr[:, k : k + 1], t[0:16, col : col + 1]
                    )

            for rep in range(reps):
                t = xpool.tile([128, IN_COLS], bf16, tag="t")
                # weights first, then data in NGRP slices so compute can
                # start as soon as the first slice lands
                nc.sync.dma_start(t[:, 0:W_COLS], in_d[:, 0:W_COLS])
                for g in range(NGRP):
                    gs = slice(W_COLS + g * GRP_COLS, W_COLS + (g + 1) * GRP_COLS)
                    nc.sync.dma_start(t[:, gs], in_d[:, gs])

                wsel = t[:, 0:SEL_COLS]
                w2p = t[:, SEL_COLS:W_COLS]

                # absorb the weights-DMA wait on PE and DVE
                absorb(0)
                if rep == 0:
                    obuf = opool.tile([L, BDC], f32, tag="obuf")
                else:
                    # Absorb the previous rep's output-DMA wait (WAR on
                    # obuf) on DVE before the block loop needs obuf.
                    nc.vector.tensor_copy(obuf[0:16, 0:1], dscr[0:16, 0:1])

                for blk in range(NBLK):
                    g, o = blk // 2, (blk % 2) * BLK
                    g0 = W_COLS + g * GRP_COLS
                    x1s = t[:, g0 + o : g0 + o + BLK]
                    x0s = t[:, g0 + 2 * BLK + o : g0 + 2 * BLK + o + BLK]
                    cs = slice(blk * BLK, (blk + 1) * BLK)
                    fused_pairs = (
                        FUSED_PAIRS_EVEN if blk % 2 == 0 else FUSED_PAIRS_ODD
                    )
                    x0b = (
                        x0s
                        .rearrange("p (a b) -> p a b", a=1)
                        .to_broadcast((128, 2, BLK))
                    )
                    if blk % 2 == 0:
                        # absorb this data-group's DMA wait on PE and DVE
                        absorb(g0)
                    opsum = opsum_pool.tile([L, BLK], f32, tag="opsum")
                    for cp in range(NCHUNK // 2):
                        zp = zpool.tile([128, 2 * BLK], f32)
                        nc.tensor.matmul(
                            zp[:, 0:BLK],
                            w2p[0:64, cp * 128 : (cp + 1) * 128],
                            x1s[0:64, :],
                            start=True,
                            stop=True,
                        )
                        nc.tensor.matmul(
                            zp[:, BLK : 2 * BLK],
                            w2p[64:128, cp * 128 : (cp + 1) * 128],
                            x1s[64:128, :],
                            start=True,
                            stop=True,
                        )
                        prod = ppool.tile([128, 2 * BLK], bf16, tag="prod")
                        pv = prod[:].rearrange("p (a b) -> p a b", a=2)
                        if cp in fused_pairs:
                            nc.vector.tensor_tensor(
                                pv,
                                zp[:].rearrange("p (a b) -> p a b", a=2),
                                x0b,
                                op=mybir.AluOpType.mult,
                            )
                        else:
                            nc.scalar.copy(prod[:], zp[:])
                            nc.vector.tensor_tensor(
                                pv, pv, x0b, op=mybir.AluOpType.mult
                            )
                        for h in range(2):
                            c = 2 * cp + h
                            nc.tensor.matmul(
                                opsum[:],
                                wsel[:, c * L : (c + 1) * L],
                                prod[:, h * BLK : (h + 1) * BLK],
                                start=(c == 0),
                                stop=(c == NCHUNK - 1),
                            )
                    nc.vector.tensor_copy(obuf[:, cs], opsum[:])

                nc.sync.dma_start(out_d[:], obuf[:])

    _strip_self_waits(nc)
    return nc


def _strip_self_waits(nc):
    """Transitively minimize semaphore waits (this container's walrus allows
    only ONE sync-wait per data instruction).

    Tile emits per-engine-minimal waits but does not track that syncing on
    engine X also conveys everything X had itself waited on.  We recompute a
    conservative happens-before: walk instructions in BIR order (a valid
    topological/issue order), maintain per-engine knowledge as a vector
    clock over semaphore values, and record, per semaphore value, the
    (joined) knowledge implied by the updating instruction's completion.
    A wait that is covered by engine knowledge plus the other kept waits is
    dropped."""
    from bass_rust import SyncInfo

    def join(a, b):
        for k, v in b.items():
            if a.get(k, 0) < v:
                a[k] = v
        return a

    def covers(k, sem, val):
        return k.get(sem, 0) >= val

    sem_cum: dict = {}
    # per-sem running joined knowledge along its event sequence:
    # list of (cum_value, knowledge_dict_at_or_before_this_value)
    sem_events: dict = {}
    engine_know: dict = {}

    # Semaphores that are ever decremented/reset (barrier gather sems) are
    # not monotone — never reason about them, never drop their waits.
    nonmono = set()
    for func in nc.m.functions:
        for blk in func.blocks:
            for inst in blk.instructions:
                si = inst.sync_info
                if si is None:
                    continue
                for upd in si.on_update:
                    if upd.update_mode not in ("sem-inc", "sem-add-imm"):
                        nonmono.add(upd.ant_name)

    def wait_knowledge(sem, val):
        """Knowledge implied by observing sem >= val."""
        k = {sem: val}
        events = sem_events.get(sem)
        if not events:
            return k
        # join knowledge of all events with cum <= observed value is already
        # accumulated (running join); take the latest event with cum <= val
        # ... but sem >= val implies all events up to the FIRST event with
        # cum >= val have completed.
        best = None
        for cum, kn in events:
            if cum >= val:
                best = kn
                break
        if best is None:
            best = events[-1][1]
        return join(dict(best), k)

    for func in nc.m.functions:
        for blk in func.blocks:
            for inst in blk.instructions:
                eng = str(inst.engine).split(".")[-1]
                know = engine_know.setdefault(eng, {})
                si = inst.sync_info
                waits = list(si.on_wait) if si is not None else []
                updates = list(si.on_update) if si is not None else []

                if waits:
                    wait_ks = [
                        {} if w.ant_name in nonmono
                        else wait_knowledge(w.ant_name, w.wait_value)
                        for w in waits
                    ]
                    # keep strongest-first waits not covered by engine
                    # knowledge + already-kept waits
                    order = sorted(range(len(waits)), key=lambda i: -len(wait_ks[i]))
                    kept, kept_ks = [], []
                    for i in order:
                        if waits[i].ant_name in nonmono:
                            kept.append(waits[i])
                            kept_ks.append(wait_ks[i])
                            continue
                        base = dict(know)
                        for kk in kept_ks:
                            join(base, kk)
                        if covers(base, waits[i].ant_name, waits[i].wait_value):
                            continue
                        kept.append(waits[i])
                        kept_ks.append(wait_ks[i])
                    # elimination pass: a kept wait may be covered by the
                    # union of the OTHER kept waits' knowledge
                    changed = True
                    while changed and len(kept) > 1:
                        changed = False
                        for i in range(len(kept)):
                            if kept[i].ant_name in nonmono:
                                continue
                            base = dict(know)
                            for j in range(len(kept)):
                                if j != i:
                                    join(base, kept_ks[j])
                            if covers(base, kept[i].ant_name, kept[i].wait_value):
                                kept.pop(i)
                                kept_ks.pop(i)
                                changed = True
                                break
                    # all original waits' knowledge is valid here (each
                    # condition holds once the kept set is satisfied)
                    for kk in wait_ks:
                        join(know, kk)
                    if len(kept) > 1:
                        raise RuntimeError(
                            f"instruction {inst.name} still has {len(kept)} "
                            f"waits: {[w.ant_name for w in kept]} "
                            f"({str(inst)[:220]})"
                        )
                    if len(kept) != len(waits):
                        inst.sync_info = SyncInfo(
                            on_wait=kept, on_update=updates
                        )

                for upd in updates:
                    s = upd.ant_name
                    if s in nonmono:
                        continue
                    sem_cum[s] = sem_cum.get(s, 0) + upd.update_value
                    post = dict(know)
                    post[s] = sem_cum[s]
                    events = sem_events.setdefault(s, [])
                    if events:
                        post = join(dict(events[-1][1]), post)
                    events.append((sem_cum[s], post))
                    # Same-engine completions are ordered: the engine's next
                    # instruction may rely on this one having finished —
                    # but ONLY for the engine's own semaphore (DMA-lane sems
                    # fire asynchronously at transfer completion).
                    if s.split("_")[0] == eng:
                        if know.get(s, 0) < sem_cum[s]:
                            know[s] = sem_cum[s]


def _prep_host(x0, x1, filters):
    import ml_dtypes

    bf16 = ml_dtypes.bfloat16

    x0 = np.asarray(x0, dtype=np.float32)
    x1 = np.asarray(x1, dtype=np.float32)
    w = np.asarray(filters, dtype=np.float32)[0]          # [F1*F2, L]

    # feature-major, (b, d) columns
    x0t = x0.transpose(1, 0, 2).reshape(F1, BD)
    x1t = x1.transpose(1, 0, 2).reshape(F2, BD)
    x0d = np.concatenate([x0t, x0t], axis=0).astype(bf16)  # [128, BD]
    x1d = np.concatenate([x1t, x1t], axis=0).astype(bf16)  # [128, BD]

    # w2li[j, l*F1 + i] = W[i*F2+j, l]
    wf = w.reshape(F1, F2, L)                             # [i, j, l]
    w2li = wf.transpose(1, 2, 0).reshape(F2, L * F1)      # [j, (l,i)]

    # chunk pairs stacked on partitions: [128, 4, 128]
    w2pair = np.empty((128, NCHUNK // 2, 128), dtype=np.float32)
    for cp in range(NCHUNK // 2):
        w2pair[0:64, cp, :] = w2li[:, (2 * cp) * 128 : (2 * cp + 1) * 128]
        w2pair[64:128, cp, :] = w2li[:, (2 * cp + 1) * 128 : (2 * cp + 2) * 128]
    w2pair = w2pair.reshape(128, W2P_COLS).astype(bf16)

    wsel = np.zeros((128, SEL_COLS), dtype=np.float32)
    for c in range(NCHUNK):
        for p in range(128):
            l = 2 * c + p // F1
            wsel[p, c * L + l] = 1.0
    wsel = wsel.astype(bf16)

    return wsel, w2pair, x1d, x0d


def _core_in_maps(inputs, reps=1):
    wsel, w2pair, x1d, x0d = _prep_host(
        inputs["x0"], inputs["x1"], inputs["filters"]
    )
    ver = np.zeros((1, VERSION * 1000 + reps), dtype=np.float32)
    in_maps = []
    for c in range(NCORES):
        parts = [wsel, w2pair]
        for g in range(NGRP):
            gs = slice(c * BDC + g * 2 * BLK, c * BDC + (g + 1) * 2 * BLK)
            parts.append(x1d[:, gs])
            parts.append(x0d[:, gs])
        inp = np.concatenate(parts, axis=1)
        in_maps.append({"inp": np.ascontiguousarray(inp), "ver": ver})
    return in_maps


def _run(inputs, trace=False):
    from concourse.bass_utils import run_bass_kernel_spmd

    if 1 not in _BASS_CACHE:
        _BASS_CACHE[1] = _build_bass(1)
    nc = _BASS_CACHE[1]

    in_maps = _core_in_maps(inputs)
    res = run_bass_kernel_spmd(nc, in_maps, list(range(NCORES)), trace=trace)

    outp = np.concatenate([res.results[c]["out"] for c in range(NCORES)], axis=1)
    # outp[l, b*D+d] -> out[b, l, d]
    out = np.ascontiguousarray(outp.reshape(L, B, D).transpose(1, 0, 2))
    return out, res


def kernel(**inputs):
    out, _ = _run(inputs, trace=False)
    return out


# ----------------------------------------------------------------------
# Benchmarking (test.py only): persistent jitted runner + in-NEFF reps.
# HW time is estimated from the wall-clock slope between reps variants,
# which cancels the per-execute RPC/launch overhead.
# ----------------------------------------------------------------------


def _make_runner(nc, in_maps):
    import jax
    import numpy as np_
    from jax.experimental.shard_map import shard_map
    from jax.sharding import Mesh, NamedSharding, PartitionSpec

    from concourse import bass2jax, mybir

    bass2jax.install_neuronx_cc_hook()

    partition_name = (
        nc.partition_id_tensor.name if nc.partition_id_tensor else None
    )
    in_names, out_names, out_avals, zero_outs = [], [], [], []
    for alloc in nc.m.functions[0].allocations:
        if not isinstance(alloc, mybir.MemoryLocationSet):
            continue
        name = alloc.memorylocations[0].name
        if alloc.kind == "ExternalInput":
            if name != partition_name:
                in_names.append(name)
        elif alloc.kind == "ExternalOutput":
            out_names.append(name)
            shape = tuple(alloc.tensor_shape)
            dtype = mybir.dt.np(alloc.dtype)
            out_avals.append(jax.core.ShapedArray(shape, dtype))
            zero_outs.append(np_.zeros(shape, dtype))

    n_params = len(in_names)
    all_names = in_names + out_names
    if partition_name is not None:
        all_names = all_names + [partition_name]
    donate = tuple(range(n_params, n_params + len(out_names)))

    def _body(*args):
        operands = list(args)
        if partition_name is not None:
            operands.append(bass2jax.partition_id_tensor())
        outs = bass2jax._bass_exec_p.bind(
            *operands,
            out_avals=tuple(out_avals),
            in_names=tuple(all_names),
            out_names=tuple(out_names),
            lowering_input_output_aliases=(),
            sim_require_finite=True,
            sim_require_nnan=True,
            nc=nc,
        )
        return tuple(outs)

    devices = jax.devices()[:NCORES]
    mesh = Mesh(np_.asarray(devices), ("core",))
    spec = PartitionSpec("core")
    in_specs = (spec,) * (n_params + len(out_names))
    out_specs = (spec,) * len(out_names)
    sharded = jax.jit(
        shard_map(
            _body, mesh=mesh, in_specs=in_specs, out_specs=out_specs, check_rep=False
        ),
        donate_argnums=donate,
        keep_unused=True,
    )

    sh = NamedSharding(mesh, spec)
    in_global = [
        jax.device_put(
            np_.concatenate([np_.asarray(m[name]) for m in in_maps], axis=0), sh
        )
        for name in in_names
    ]
    zeros_np = [
        np_.zeros((NCORES * z.shape[0], *z.shape[1:]), z.dtype) for z in zero_outs
    ]

    def call():
        zeros_dev = [jax.device_put(z, sh) for z in zeros_np]
        jax.block_until_ready(zeros_dev)
        import time

        t0 = time.perf_counter()
        out = sharded(*in_global, *zeros_dev)
        jax.block_until_ready(out)
        t1 = time.perf_counter()
        return (t1 - t0), out

    return call


def bench(inputs, reps_pair=(1, 65), n_timed=20):
    mins = {}
    raw = {}
    for reps in reps_pair:
        in_maps = _core_in_maps(inputs, reps)
        if reps not in _BASS_CACHE:
            _BASS_CACHE[reps] = _build_bass(reps)
        call = _make_runner(_BASS_CACHE[reps], in_maps)
        for _ in range(3):
            call()  # warmup (compile + caches)
        times = [call()[0] for _ in range(n_timed)]
        mins[reps] = min(times)
        raw[reps] = sorted(times)[:5]

    r0, r1 = reps_pair
    per_rep_ns = (mins[r1] - mins[r0]) / (r1 - r0) * 1e9
    return per_rep_ns, mins, raw


# revision 44
# speedup vs baseline: 6.2399x; 1.3868x over previous
"""Trainium2 Bass kernel for nn_ExtremeFMLayer.

Math:  out[b,l,d] = sum_{i,j} W[i*F2+j, l] * x0[b,i,d] * x1[b,j,d]
  (B, F1, F2, D, L) = (2048, 64, 64, 16, 16)

Mapping (per core, data-parallel over batch, bd = flattened (b, d) columns):
  stage 1 (PE):   Z[(l,i), bd]   = sum_j W2li[j, (l,i)] * x1t[j, bd]
                  K=64 row-packed: two chunks run concurrently in array
                  rows 0-63 / 64-127 (tile_position via base_partition).
  stage 2:        P[(l,i), bd]   = Z[(l,i), bd] * x0t[i, bd]
                  split across engines:
                    fused path:  DVE tensor_mul PSUM(fp32) x SBUF(bf16) -> bf16
                    ACT path:    ScalarE copies PSUM -> SBUF bf16, DVE
                                 multiplies in place at 2x bf16 rate
  stage 3 (PE):   out[l, bd]    = selector GEMM over (l,i) chunk partitions
                                  (0/1 weights, accumulated in PSUM)

All inputs ship as ONE bf16 [128, 640 + 2*BDC] tensor per core:
  [ wsel(128) | w2pairs(512) | x1 stacked twice(BDC) | x0 stacked twice(BDC) ]

The walrus build here allows only ONE sync-wait per data instruction; the
structure (single input DMA, DVE absorber, in-place TT, one-semaphore
eviction chains) keeps every instruction at <=1 wait, with a post-pass
stripping provably redundant waits.
"""

import sys

if "/opt/trn_rl_repo" not in sys.path:
    sys.path.insert(0, "/opt/trn_rl_repo")

import numpy as np

B, F1, F2, D, L = 2048, 64, 64, 16, 16
NCORES = 8
BD = B * D                  # 32768
BDC = BD // NCORES          # 4096 columns per core
NBLK = 8
BLK = BDC // NBLK           # 512
NCHUNK = 8                  # (l,i) chunks of 128 rows

SEL_COLS = NCHUNK * L       # 128
W2P_COLS = (NCHUNK // 2) * 128  # 512 (chunk pairs stacked on partitions)
W_COLS = SEL_COLS + W2P_COLS    # 640
NGRP = 4                    # data shipped as 4 DMAs of 2 blocks each
GRP_COLS = 2 * BLK * 2      # x1 pair-of-blocks + x0 pair-of-blocks = 2048
IN_COLS = W_COLS + NGRP * GRP_COLS

# chunk-pairs handled by the fused DVE path, per block parity (engine balance)
FUSED_PAIRS_EVEN = (0,)
FUSED_PAIRS_ODD = ()

_BASS_CACHE: dict = {}

# Bumped on every kernel change: the persistent NEFF compile cache keys on
# the HLO (shapes/names only, not BIR contents), so a shape-unique dummy
# input is needed to keep kernel variants from silently reusing each
# other's NEFFs.
VERSION = 42
PROBE_SKIP_SEL = False  # timing probe: halve PE selector work (wrong output)


def _build_bass(reps=1):
    from concourse import bass, tile
    from concourse import mybir

    f32 = mybir.dt.float32
    bf16 = mybir.dt.bfloat16
    nc = bass.Bass()

    in_d = nc.declare_dram_parameter("inp", [128, IN_COLS], bf16, isOutput=False)
    nc.declare_dram_parameter("ver", [1, VERSION * 1000 + reps], f32, isOutput=False)
    out_d = nc.declare_dram_parameter("out", [L, BDC], f32, isOutput=True)

    with tile.TileContext(nc) as tc:
        with (
            tc.tile_pool(name="const", bufs=1) as cpool,
            tc.tile_pool(name="xin", bufs=2) as xpool,
            tc.tile_pool(name="prod", bufs=10) as ppool,
            tc.tile_pool(name="outb", bufs=1) as opool,
            tc.tile_pool(name="zpsum", bufs=3, space=bass.MemorySpace.PSUM) as zpool,
            tc.tile_pool(name="opsum", bufs=2, space=bass.MemorySpace.PSUM) as opsum_pool,
        ):
            dscr = cpool.tile([16, 512], bf16)
            obuf = None
            absorb_idx = [0]

            def absorb(col, pe=True, dve=True):
                # Tiny ops that make PE/DVE observe a DMA-completion wait
                # early, so real instructions carry at most one wait.
                # PE: a dummy LDWEIGHTS (no PSUM write, overwritten by the
                # next matmul's own weight load).  DVE: a 1-column copy to
                # a distinct dscr column (race-free).
                k = absorb_idx[0]
                absorb_idx[0] += 1
                if pe:
                    nc.tensor.ldweights(t[0:1, col : col + 1])
                if dve:
                    nc.vector.tensor_copy(
                        dscr[:, k : k + 1], t[0:16, col : col + 1]
                    )

            for rep in range(reps):
                t = xpool.tile([128, IN_COLS], bf16, tag="t")
                # weights first, then data in NGRP slices so compute can
                # start as soon as the first slice lands
                nc.sync.dma_start(t[:, 0:W_COLS], in_d[:, 0:W_COLS])
                for g in range(NGRP):
                    gs = slice(W_COLS + g * GRP_COLS, W_COLS + (g + 1) * GRP_COLS)
                    nc.sync.dma_start(t[:, gs], in_d[:, gs])

                wsel = t[:, 0:SEL_COLS]
                w2p = t[:, SEL_COLS:W_COLS]

                # absorb the weights-DMA wait on PE and DVE
                absorb(0)
                if rep == 0:
                    obuf = opool.tile([L, BDC], f32, tag="obuf")
                else:
                    # Absorb the previous rep's output-DMA wait (WAR on
                    # obuf) on DVE before the block loop needs obuf.
                    nc.vector.tensor_copy(obuf[0:16, 0:1], dscr[0:16, 0:1])

                def emit_sel(st):
                    # selector GEMM for a completed block (one lag behind,
                    # so the PSUM->evict->multiply round trip never stalls
                    # the PE queue)
                    prods, opsum_p, cs_p = st
                    for c in range(NCHUNK):
                        if PROBE_SKIP_SEL and c % 2 == 1:
                            continue
                        nc.tensor.matmul(
                            opsum_p[:],
                            wsel[:, c * L : (c + 1) * L],
                            prods[c // 2][:, (c % 2) * BLK : (c % 2 + 1) * BLK],
                            start=(c == 0),
                            stop=(
                                c == (NCHUNK - 2 if PROBE_SKIP_SEL else NCHUNK - 1)
                            ),
                        )
                    nc.vector.tensor_copy(obuf[:, cs_p], opsum_p[:])

                pending = None
                for blk in range(NBLK):
                    g, o = blk // 2, (blk % 2) * BLK
                    g0 = W_COLS + g * GRP_COLS
                    x1s = t[:, g0 + o : g0 + o + BLK]
                    x0s = t[:, g0 + 2 * BLK + o : g0 + 2 * BLK + o + BLK]
                    cs = slice(blk * BLK, (blk + 1) * BLK)
                    fused_pairs = (
                        FUSED_PAIRS_EVEN if blk % 2 == 0 else FUSED_PAIRS_ODD
                    )
                    x0b = (
                        x0s
                        .rearrange("p (a b) -> p a b", a=1)
                        .to_broadcast((128, 2, BLK))
                    )
                    if blk % 2 == 0:
                        # absorb this data-group's DMA wait on PE and DVE
                        absorb(g0)
                    opsum = opsum_pool.tile([L, BLK], f32, tag="opsum")
                    prods = []
                    for cp in range(NCHUNK // 2):
                        zp = zpool.tile([128, 2 * BLK], f32)
                        nc.tensor.matmul(
                            zp[:, 0:BLK],
                            w2p[0:64, cp * 128 : (cp + 1) * 128],
                            x1s[0:64, :],
                            start=True,
                            stop=True,
                        )
                        nc.tensor.matmul(
                            zp[:, BLK : 2 * BLK],
                            w2p[64:128, cp * 128 : (cp + 1) * 128],
                            x1s[64:128, :],
                            start=True,
                            stop=True,
                        )
                        prod = ppool.tile([128, 2 * BLK], bf16, tag="prod")
                        pv = prod[:].rearrange("p (a b) -> p a b", a=2)
                        if cp in fused_pairs:
                            nc.vector.tensor_tensor(
                                pv,
                                zp[:].rearrange("p (a b) -> p a b", a=2),
                                x0b,
                                op=mybir.AluOpType.mult,
                            )
                        else:
                            nc.scalar.copy(prod[:], zp[:])
                            nc.vector.tensor_tensor(
                                pv, pv, x0b, op=mybir.AluOpType.mult
                            )
                        prods.append(prod)
                    if pending is not None:
                        emit_sel(pending)
                    pending = (prods, opsum, cs)
                emit_sel(pending)

                nc.sync.dma_start(out_d[:], obuf[:])

    _strip_self_waits(nc)
    return nc


def _strip_self_waits(nc):
    """Transitively minimize semaphore waits (this container's walrus allows
    only ONE sync-wait per data instruction).

    Tile emits per-engine-minimal waits but does not track that syncing on
    engine X also conveys everything X had itself waited on.  We recompute a
    conservative happens-before: walk instructions in BIR order (a valid
    topological/issue order), maintain per-engine knowledge as a vector
    clock over semaphore values, and record, per semaphore value, the
    (joined) knowledge implied by the updating instruction's completion.
    A wait that is covered by engine knowledge plus the other kept waits is
    dropped."""
    from bass_rust import SyncInfo

    def join(a, b):
        for k, v in b.items():
            if a.get(k, 0) < v:
                a[k] = v
        return a

    def covers(k, sem, val):
        return k.get(sem, 0) >= val

    sem_cum: dict = {}
    # per-sem running joined knowledge along its event sequence:
    # list of (cum_value, knowledge_dict_at_or_before_this_value)
    sem_events: dict = {}
    engine_know: dict = {}

    # Semaphores that are ever decremented/reset (barrier gather sems) are
    # not monotone — never reason about them, never drop their waits.
    nonmono = set()
    for func in nc.m.functions:
        for blk in func.blocks:
            for inst in blk.instructions:
                si = inst.sync_info
                if si is None:
                    continue
                for upd in si.on_update:
                    if upd.update_mode not in ("sem-inc", "sem-add-imm"):
                        nonmono.add(upd.ant_name)

    def wait_knowledge(sem, val):
        """Knowledge implied by observing sem >= val."""
        k = {sem: val}
        events = sem_events.get(sem)
        if not events:
            return k
        # join knowledge of all events with cum <= observed value is already
        # accumulated (running join); take the latest event with cum <= val
        # ... but sem >= val implies all events up to the FIRST event with
        # cum >= val have completed.
        best = None
        for cum, kn in events:
            if cum >= val:
                best = kn
                break
        if best is None:
            best = events[-1][1]
        return join(dict(best), k)

    for func in nc.m.functions:
        for blk in func.blocks:
            for inst in blk.instructions:
                eng = str(inst.engine).split(".")[-1]
                know = engine_know.setdefault(eng, {})
                si = inst.sync_info
                waits = list(si.on_wait) if si is not None else []
                updates = list(si.on_update) if si is not None else []

                if waits:
                    wait_ks = [
                        {} if w.ant_name in nonmono
                        else wait_knowledge(w.ant_name, w.wait_value)
                        for w in waits
                    ]
                    # keep strongest-first waits not covered by engine
                    # knowledge + already-kept waits
                    order = sorted(range(len(waits)), key=lambda i: -len(wait_ks[i]))
                    kept, kept_ks = [], []
                    for i in order:
                        if waits[i].ant_name in nonmono:
                            kept.append(waits[i])
                            kept_ks.append(wait_ks[i])
                            continue
                        base = dict(know)
                        for kk in kept_ks:
                            join(base, kk)
                        if covers(base, waits[i].ant_name, waits[i].wait_value):
                            continue
                        kept.append(waits[i])
                        kept_ks.append(wait_ks[i])
                    # elimination pass: a kept wait may be covered by the
                    # union of the OTHER kept waits' knowledge
                    changed = True
                    while changed and len(kept) > 1:
                        changed = False
                        for i in range(len(kept)):
                            if kept[i].ant_name in nonmono:
                                continue
                            base = dict(know)
                            for j in range(len(kept)):
                                if j != i:
                                    join(base, kept_ks[j])
                            if covers(base, kept[i].ant_name, kept[i].wait_value):
                                kept.pop(i)
                                kept_ks.pop(i)
                                changed = True
                                break
                    # all original waits' knowledge is valid here (each
                    # condition holds once the kept set is satisfied)
                    for kk in wait_ks:
                        join(know, kk)
                    if len(kept) > 1:
                        raise RuntimeError(
                            f"instruction {inst.name} still has {len(kept)} "
                            f"waits: {[w.ant_name for w in kept]} "
                            f"({str(inst)[:220]})"
                        )
                    if len(kept) != len(waits):
                        inst.sync_info = SyncInfo(
                            on_wait=kept, on_update=updates
                        )

                for upd in updates:
                    s = upd.ant_name
                    if s in nonmono:
                        continue
                    sem_cum[s] = sem_cum.get(s, 0) + upd.update_value
                    post = dict(know)
                    post[s] = sem_cum[s]
                    events = sem_events.setdefault(s, [])
                    if events:
                        post = join(dict(events[-1][1]), post)
                    events.append((sem_cum[s], post))
                    # Same-engine completions are ordered: the engine's next
                    # instruction may rely on this one having finished —
                    # but ONLY for the engine's own semaphore (DMA-lane sems
                    # fire asynchronously at transfer completion).
                    if s.split("_")[0] == eng:
                        if know.get(s, 0) < sem_cum[s]:
                            know[s] = sem_cum[s]


def _prep_host(x0, x1, filters):
    import ml_dtypes

    bf16 = ml_dtypes.bfloat16

    x0 = np.asarray(x0, dtype=np.float32)
    x1 = np.asarray(x1, dtype=np.float32)
    w = np.asarray(filters, dtype=np.float32)[0]          # [F1*F2, L]

    # feature-major, (b, d) columns
    x0t = x0.transpose(1, 0, 2).reshape(F1, BD)
    x1t = x1.transpose(1, 0, 2).reshape(F2, BD)
    x0d = np.concatenate([x0t, x0t], axis=0).astype(bf16)  # [128, BD]
    x1d = np.concatenate([x1t, x1t], axis=0).astype(bf16)  # [128, BD]

    # w2li[j, l*F1 + i] = W[i*F2+j, l]
    wf = w.reshape(F1, F2, L)                             # [i, j, l]
    w2li = wf.transpose(1, 2, 0).reshape(F2, L * F1)      # [j, (l,i)]

    # chunk pairs stacked on partitions: [128, 4, 128]
    w2pair = np.empty((128, NCHUNK // 2, 128), dtype=np.float32)
    for cp in range(NCHUNK // 2):
        w2pair[0:64, cp, :] = w2li[:, (2 * cp) * 128 : (2 * cp + 1) * 128]
        w2pair[64:128, cp, :] = w2li[:, (2 * cp + 1) * 128 : (2 * cp + 2) * 128]
    w2pair = w2pair.reshape(128, W2P_COLS).astype(bf16)

    wsel = np.zeros((128, SEL_COLS), dtype=np.float32)
    for c in range(NCHUNK):
        for p in range(128):
            l = 2 * c + p // F1
            wsel[p, c * L + l] = 1.0
    wsel = wsel.astype(bf16)

    return wsel, w2pair, x1d, x0d


def _core_in_maps(inputs, reps=1):
    wsel, w2pair, x1d, x0d = _prep_host(
        inputs["x0"], inputs["x1"], inputs["filters"]
    )
    ver = np.zeros((1, VERSION * 1000 + reps), dtype=np.float32)
    in_maps = []
    for c in range(NCORES):
        parts = [wsel, w2pair]
        for g in range(NGRP):
            gs = slice(c * BDC + g * 2 * BLK, c * BDC + (g + 1) * 2 * BLK)
            parts.append(x1d[:, gs])
            parts.append(x0d[:, gs])
        inp = np.concatenate(parts, axis=1)
        in_maps.append({"inp": np.ascontiguousarray(inp), "ver": ver})
    return in_maps


def _run(inputs, trace=False):
    from concourse.bass_utils import run_bass_kernel_spmd

    if 1 not in _BASS_CACHE:
        _BASS_CACHE[1] = _build_bass(1)
    nc = _BASS_CACHE[1]

    in_maps = _core_in_maps(inputs)
    res = run_bass_kernel_spmd(nc, in_maps, list(range(NCORES)), trace=trace)

    outp = np.concatenate([res.results[c]["out"] for c in range(NCORES)], axis=1)
    # outp[l, b*D+d] -> out[b, l, d]
    out = np.ascontiguousarray(outp.reshape(L, B, D).transpose(1, 0, 2))
    return out, res


def kernel(**inputs):
    out, _ = _run(inputs, trace=False)
    return out


# ----------------------------------------------------------------------
# Benchmarking (test.py only): persistent jitted runner + in-NEFF reps.
# HW time is estimated from the wall-clock slope between reps variants,
# which cancels the per-execute RPC/launch overhead.
# ----------------------------------------------------------------------


def _make_runner(nc, in_maps):
    import jax
    import numpy as np_
    from jax.experimental.shard_map import shard_map
    from jax.sharding import Mesh, NamedSharding, PartitionSpec

    from concourse import bass2jax, mybir

    bass2jax.install_neuronx_cc_hook()

    partition_name = (
        nc.partition_id_tensor.name if nc.partition_id_tensor else None
    )
    in_names, out_names, out_avals, zero_outs = [], [], [], []
    for alloc in nc.m.functions[0].allocations:
        if not isinstance(alloc, mybir.MemoryLocationSet):
            continue
        name = alloc.memorylocations[0].name
        if alloc.kind == "ExternalInput":
            if name != partition_name:
                in_names.append(name)
        elif alloc.kind == "ExternalOutput":
            out_names.append(name)
            shape = tuple(alloc.tensor_shape)
            dtype = mybir.dt.np(alloc.dtype)
            out_avals.append(jax.core.ShapedArray(shape, dtype))
            zero_outs.append(np_.zeros(shape, dtype))

    n_params = len(in_names)
    all_names = in_names + out_names
    if partition_name is not None:
        all_names = all_names + [partition_name]
    donate = tuple(range(n_params, n_params + len(out_names)))

    def _body(*args):
        operands = list(args)
        if partition_name is not None:
            operands.append(bass2jax.partition_id_tensor())
        outs = bass2jax._bass_exec_p.bind(
            *operands,
            out_avals=tuple(out_avals),
            in_names=tuple(all_names),
            out_names=tuple(out_names),
            lowering_input_output_aliases=(),
            sim_require_finite=True,
            sim_require_nnan=True,
            nc=nc,
        )
        return tuple(outs)

    devices = jax.devices()[:NCORES]
    mesh = Mesh(np_.asarray(devices), ("core",))
    spec = PartitionSpec("core")
    in_specs = (spec,) * (n_params + len(out_names))
    out_specs = (spec,) * len(out_names)
    sharded = jax.jit(
        shard_map(
            _body, mesh=mesh, in_specs=in_specs, out_specs=out_specs, check_rep=False
        ),
        donate_argnums=donate,
        keep_unused=True,
    )

    sh = NamedSharding(mesh, spec)
    in_global = [
        jax.device_put(
            np_.concatenate([np_.asarray(m[name]) for m in in_maps], axis=0), sh
        )
        for name in in_names
    ]
    zeros_np = [
        np_.zeros((NCORES * z.shape[0], *z.shape[1:]), z.dtype) for z in zero_outs
    ]

    def call():
        zeros_dev = [jax.device_put(z, sh) for z in zeros_np]
        jax.block_until_ready(zeros_dev)
        import time

        t0 = time.perf_counter()
        out = sharded(*in_global, *zeros_dev)
        jax.block_until_ready(out)
        t1 = time.perf_counter()
        return (t1 - t0), out

    return call


def bench(inputs, reps_pair=(1, 65), n_timed=20):
    mins = {}
    raw = {}
    for reps in reps_pair:
        in_maps = _core_in_maps(inputs, reps)
        if reps not in _BASS_CACHE:
            _BASS_CACHE[reps] = _build_bass(reps)
        call = _make_runner(_BASS_CACHE[reps], in_maps)
        for _ in range(3):
            call()  # warmup (compile + caches)
        times = [call()[0] for _ in range(n_timed)]
        mins[reps] = min(times)
        raw[reps] = sorted(times)[:5]

    r0, r1 = reps_pair
    per_rep_ns = (mins[r1] - mins[r0]) / (r1 - r0) * 1e9
    return per_rep_ns, mins, raw
